# revision 6
# baseline (speedup 1.0000x reference)
"""Trainium2 Bass kernel for strided Conv2d + stride-permutation + bias.

Problem (hardcoded):
  x      [16, 256, 64, 64] f32
  weight [256, 256, 3, 3]  f32  (OIHW)
  bias   [256]             f32
  conv: stride (2,2), padding (1,1), dilation (1,1) -> [16, 256, 32, 32]
  output: spatial flattened and permuted into the 4 stride-phase groups
          (si, sj, i, j) order, + bias -> [16, 256, 1024]

Strategy: data-parallel over batch across 8 cores (2 images/core).
Per core the conv is computed as 18 accumulating matmuls per PSUM group
(2 ci-tiles x 9 taps), contracting ci (128 partitions) with the 3x3 tap
positions addressed via phase-split padded input planes:
  x is split on host into 4 parity planes per (image, ci-tile)
  [(row%2, col%2) -> 34x34 zero-padded plane], so every tap's rhs is a
  simple 2D strided slice with unit-stride columns.
Outputs accumulate in PSUM [co=128, 16x32]; ScalarE evicts with bias-add
while scattering into the stride-permuted output layout, which makes the
final DMA to HBM fully contiguous.
"""

import os
import time

import numpy as np

_B, _C, _H, _W = 16, 256, 64, 64
_HO = _WO = 32
_NCORES = 8
_IMGS = _B // _NCORES  # images per core
_PL = 34  # padded phase-plane side
_PLSZ = _PL * _PL

# tap index (0,1,2) -> (row/col phase, start offset in padded plane)
_TAP = {0: (1, 0), 1: (0, 1), 2: (1, 1)}

# taps ordered by phase-plane DMA arrival order (ph3, ph2, ph1, ph0)
_TAP_ORDER = [
    (0, 0), (0, 2), (2, 0), (2, 2),  # phase (1,1) = plane 3
    (0, 1), (2, 1),                  # phase (1,0) = plane 2
    (1, 0), (1, 2),                  # phase (0,1) = plane 1
    (1, 1),                          # phase (0,0) = plane 0
]

_PROG_CACHE = {}


def _build_program(reps: int):
    import concourse.tile as tile
    from concourse import bacc, mybir

    f32 = mybir.dt.float32
    f32r = mybir.dt.float32r

    nc = bacc.Bacc("TRN2", target_bir_lowering=False, debug=False)

    xph = nc.dram_tensor(
        "xph", [_IMGS, 2, 128, 4, _PL, _PL], f32r, kind="ExternalInput"
    ).ap()
    wt = nc.dram_tensor("wt", [128, 4608], f32r, kind="ExternalInput").ap()
    bs = nc.dram_tensor("bs", [128, 2], f32, kind="ExternalInput").ap()
    out = nc.dram_tensor("out", [_IMGS, 2, 128, 1024], f32, kind="ExternalOutput").ap()

    with tile.TileContext(nc) as tc:
        with (
            tc.tile_pool(name="const", bufs=1) as constp,
            tc.tile_pool(name="xbuf", bufs=1) as xp,
            tc.tile_pool(name="obuf", bufs=2) as obp,
            tc.tile_pool(name="psum", bufs=8, space="PSUM") as psp,
        ):
            wtile = constp.tile([128, 4608], f32r)
            btile = constp.tile([128, 2], f32)
            # weights for the first matmul group first, then bias, then rest
            nc.sync.dma_start(wtile[:, 0:1152], wt[:, 0:1152])
            nc.sync.dma_start(btile[:], bs[:])

            xt = {}
            for img in range(_IMGS):
                for cit in range(2):
                    t = xp.tile([128, 4, _PL, _PL], f32r, tag=f"x_{img}_{cit}")
                    xt[(img, cit)] = t
                    # per-plane DMAs, phase 3 first (used by most taps)
                    for ph in (3, 2, 1, 0):
                        nc.sync.dma_start(t[:, ph], xph[img, cit, :, ph])
                    if img == 0 and cit == 0:
                        for blk in range(1, 4):
                            s = blk * 1152
                            nc.sync.dma_start(
                                wtile[:, s : s + 1152], wt[:, s : s + 1152]
                            )

            for _rep in range(reps):
                for img in range(_IMGS):
                    for cot in range(2):
                        ob = obp.tile([128, 2, 2, 16, 16], f32, tag="ob")
                        for half in range(2):
                            ps = psp.tile([128, 16, 32], f32, tag="ps")
                            n = 0
                            for cit in range(2):
                                for kh, kw in _TAP_ORDER:
                                    phr, r0 = _TAP[kh]
                                    phc, c0 = _TAP[kw]
                                    rhs = xt[(img, cit)][
                                        :,
                                        phr * 2 + phc,
                                        r0 + half * 16 : r0 + half * 16 + 16,
                                        c0 : c0 + 32,
                                    ]
                                    s = cit * 2304 + (cot * 9 + kh * 3 + kw) * 128
                                    lhsT = wtile[:, s : s + 128]
                                    nc.tensor.matmul(
                                        ps[:],
                                        lhsT,
                                        rhs,
                                        start=(n == 0),
                                        stop=(n == 17),
                                    )
                                    n += 1
                            # evict PSUM -> SBUF with bias add, scattering
                            # rows/cols into the stride-permuted layout
                            for si in range(2):
                                src = ps[:, si : 16 : 2, :]  # (rh 8, c 32)
                                dst = ob[
                                    :, si, :, half * 8 : half * 8 + 8, :
                                ].rearrange("p sj rh j -> p rh j sj")
                                nc.scalar.activation(
                                    dst,
                                    src,
                                    mybir.ActivationFunctionType.Identity,
                                    bias=btile[:, cot : cot + 1],
                                )
                        nc.sync.dma_start(out[img, cot], ob[:])

    nc.compile()
    return nc


def _get_program(reps: int):
    if reps not in _PROG_CACHE:
        _PROG_CACHE[reps] = _build_program(reps)
    return _PROG_CACHE[reps]


def _round_fp32r(a):
    """Round fp32 to the fp32r format (8-bit exp, 11-bit mantissa, top-20-bit
    aligned): round-to-nearest-even on the low 12 bits."""
    u = a.astype(np.float32).view(np.uint32)
    r = (u + 0x7FF + ((u >> 12) & 1)) & np.uint32(0xFFFFF000)
    return r.view(np.float32)


def _prep_inputs(x, weight, bias):
    x = _round_fp32r(np.ascontiguousarray(np.asarray(x, dtype=np.float32)))
    weight = _round_fp32r(
        np.ascontiguousarray(np.asarray(weight, dtype=np.float32))
    )
    bias = np.ascontiguousarray(np.asarray(bias, dtype=np.float32))

    # phase-split + pad: [B, 2(cit), 128, 4(ph), 34, 34]
    xphase = np.zeros((_B, 2, 128, 4, _PL, _PL), dtype=np.float32)
    xr = x.reshape(_B, 2, 128, _H, _W)
    for rp in range(2):
        for cp in range(2):
            xphase[:, :, :, rp * 2 + cp, 1:33, 1:33] = xr[:, :, :, rp::2, cp::2]

    # weight -> lhsT layout [cip, cit*2304 + (cot*9 + tap)*128 + cop]
    w6 = weight.reshape(2, 128, 2, 128, 3, 3)  # [cot, cop, cit, cip, kh, kw]
    wt = np.ascontiguousarray(
        w6.transpose(3, 2, 0, 4, 5, 1).reshape(128, 4608)
    )  # [cip][cit, cot, kh, kw, cop]

    bs = np.ascontiguousarray(bias.reshape(2, 128).T)  # [cop, cot]

    in_maps = []
    for c in range(_NCORES):
        in_maps.append(
            {
                "xph": np.ascontiguousarray(xphase[c * _IMGS : (c + 1) * _IMGS]),
                "wt": wt,
                "bs": bs,
            }
        )
    return in_maps


def _run(in_maps, reps: int):
    from concourse.bass_utils import run_bass_kernel_spmd

    nc = _get_program(reps)
    t0 = time.perf_counter()
    res = run_bass_kernel_spmd(nc, in_maps, list(range(_NCORES)))
    dt = time.perf_counter() - t0
    outs = [res.results[c]["out"].reshape(_IMGS, _C, 1024) for c in range(_NCORES)]
    return np.concatenate(outs, axis=0), dt


def kernel(x, weight, bias):
    in_maps = _prep_inputs(x, weight, bias)
    reps = int(os.environ.get("BASS_CONV_REPS", "1"))
    out, _ = _run(in_maps, reps)
    return out


# revision 7
# speedup vs baseline: 1131.0729x; 1131.0729x over previous
"""Trainium2 Bass kernel for strided Conv2d + stride-permutation + bias.

Problem (hardcoded):
  x      [16, 256, 64, 64] f32
  weight [256, 256, 3, 3]  f32  (OIHW)
  bias   [256]             f32
  conv: stride (2,2), padding (1,1), dilation (1,1) -> [16, 256, 32, 32]
  output: spatial flattened and permuted into the 4 stride-phase groups
          (si, sj, i, j) order, + bias -> [16, 256, 1024]

Strategy: data-parallel over batch across 8 cores (2 images/core).
Per core the conv is computed as 18 accumulating matmuls per PSUM group
(2 ci-tiles x 9 taps), contracting ci (128 partitions) with the 3x3 tap
positions addressed via phase-split padded input planes:
  x is split on host into 4 parity planes per (image, ci-tile)
  [(row%2, col%2) -> 34x34 zero-padded plane], so every tap's rhs is a
  simple 2D strided slice with unit-stride columns.
Outputs accumulate in PSUM [co=128, 16x32]; ScalarE evicts with bias-add
while scattering into the stride-permuted output layout, which makes the
final DMA to HBM fully contiguous.
"""

import os
import time

import numpy as np

_B, _C, _H, _W = 16, 256, 64, 64
_HO = _WO = 32
_NCORES = 8
_IMGS = _B // _NCORES  # images per core
_PL = 34  # padded phase-plane side
_PLSZ = _PL * _PL

# tap index (0,1,2) -> (row/col phase, start offset in padded plane)
_TAP = {0: (1, 0), 1: (0, 1), 2: (1, 1)}

# taps ordered by phase-plane DMA arrival order (ph3, ph2, ph1, ph0)
_TAP_ORDER = [
    (0, 0), (0, 2), (2, 0), (2, 2),  # phase (1,1) = plane 3
    (0, 1), (2, 1),                  # phase (1,0) = plane 2
    (1, 0), (1, 2),                  # phase (0,1) = plane 1
    (1, 1),                          # phase (0,0) = plane 0
]

_PROG_CACHE = {}


def _build_program(reps: int):
    import concourse.tile as tile
    from concourse import bacc, mybir

    f32 = mybir.dt.float32
    f32r = mybir.dt.float32r

    nc = bacc.Bacc("TRN2", target_bir_lowering=False, debug=False)

    xph = nc.dram_tensor(
        "xph", [_IMGS, 2, 128, 4, _PL, _PL], f32r, kind="ExternalInput"
    ).ap()
    wt = nc.dram_tensor("wt", [128, 4608], f32r, kind="ExternalInput").ap()
    bs = nc.dram_tensor("bs", [128, 2], f32, kind="ExternalInput").ap()
    out = nc.dram_tensor("out", [_IMGS, 2, 128, 1024], f32, kind="ExternalOutput").ap()

    with tile.TileContext(nc) as tc:
        with (
            tc.tile_pool(name="const", bufs=1) as constp,
            tc.tile_pool(name="xbuf", bufs=1) as xp,
            tc.tile_pool(name="obuf", bufs=2) as obp,
            tc.tile_pool(name="psum", bufs=8, space="PSUM") as psp,
        ):
            wtile = constp.tile([128, 4608], f32r)
            btile = constp.tile([128, 2], f32)
            # weights for the first matmul group first, then bias, then rest
            nc.sync.dma_start(wtile[:, 0:1152], wt[:, 0:1152])
            nc.sync.dma_start(btile[:], bs[:])

            xt = {}
            for img in range(_IMGS):
                for cit in range(2):
                    t = xp.tile([128, 4, _PL, _PL], f32r, tag=f"x_{img}_{cit}")
                    xt[(img, cit)] = t
                    # per-plane DMAs, phase 3 first (used by most taps)
                    for ph in (3, 2, 1, 0):
                        nc.sync.dma_start(t[:, ph], xph[img, cit, :, ph])
                    if img == 0 and cit == 0:
                        for blk in range(1, 4):
                            s = blk * 1152
                            nc.sync.dma_start(
                                wtile[:, s : s + 1152], wt[:, s : s + 1152]
                            )

            for _rep in range(reps):
                for img in range(_IMGS):
                    for cot in range(2):
                        ob = obp.tile([128, 2, 2, 16, 16], f32, tag="ob")
                        for half in range(2):
                            ps = psp.tile([128, 16, 32], f32, tag="ps")
                            n = 0
                            for cit in range(2):
                                for kh, kw in _TAP_ORDER:
                                    phr, r0 = _TAP[kh]
                                    phc, c0 = _TAP[kw]
                                    rhs = xt[(img, cit)][
                                        :,
                                        phr * 2 + phc,
                                        r0 + half * 16 : r0 + half * 16 + 16,
                                        c0 : c0 + 32,
                                    ]
                                    s = cit * 2304 + (cot * 9 + kh * 3 + kw) * 128
                                    lhsT = wtile[:, s : s + 128]
                                    nc.tensor.matmul(
                                        ps[:],
                                        lhsT,
                                        rhs,
                                        start=(n == 0),
                                        stop=(n == 17),
                                    )
                                    n += 1
                            # evict PSUM -> SBUF with bias add, scattering
                            # rows/cols into the stride-permuted layout
                            for si in range(2):
                                src = ps[:, si : 16 : 2, :]  # (rh 8, c 32)
                                dst = ob[
                                    :, si, :, half * 8 : half * 8 + 8, :
                                ].rearrange("p sj rh j -> p rh j sj")
                                nc.scalar.activation(
                                    dst,
                                    src,
                                    mybir.ActivationFunctionType.Identity,
                                    bias=btile[:, cot : cot + 1],
                                )
                        nc.sync.dma_start(out[img, cot], ob[:])

    nc.compile()
    return nc


def _get_program(reps: int):
    if reps not in _PROG_CACHE:
        _PROG_CACHE[reps] = _build_program(reps)
    return _PROG_CACHE[reps]


def _round_fp32r(a):
    """Round fp32 to the fp32r format (8-bit exp, 11-bit mantissa, top-20-bit
    aligned): round-to-nearest-even on the low 12 bits."""
    u = a.astype(np.float32).view(np.uint32)
    r = (u + 0x7FF + ((u >> 12) & 1)) & np.uint32(0xFFFFF000)
    return r.view(np.float32)


def _prep_inputs(x, weight, bias):
    x = _round_fp32r(np.ascontiguousarray(np.asarray(x, dtype=np.float32)))
    weight = _round_fp32r(
        np.ascontiguousarray(np.asarray(weight, dtype=np.float32))
    )
    bias = np.ascontiguousarray(np.asarray(bias, dtype=np.float32))

    # phase-split + pad: [B, 2(cit), 128, 4(ph), 34, 34]
    xphase = np.zeros((_B, 2, 128, 4, _PL, _PL), dtype=np.float32)
    xr = x.reshape(_B, 2, 128, _H, _W)
    for rp in range(2):
        for cp in range(2):
            xphase[:, :, :, rp * 2 + cp, 1:33, 1:33] = xr[:, :, :, rp::2, cp::2]

    # weight -> lhsT layout [cip, cit*2304 + (cot*9 + tap)*128 + cop]
    w6 = weight.reshape(2, 128, 2, 128, 3, 3)  # [cot, cop, cit, cip, kh, kw]
    wt = np.ascontiguousarray(
        w6.transpose(3, 2, 0, 4, 5, 1).reshape(128, 4608)
    )  # [cip][cit, cot, kh, kw, cop]

    bs = np.ascontiguousarray(bias.reshape(2, 128).T)  # [cop, cot]

    in_maps = []
    for c in range(_NCORES):
        in_maps.append(
            {
                "xph": np.ascontiguousarray(xphase[c * _IMGS : (c + 1) * _IMGS]),
                "wt": wt,
                "bs": bs,
            }
        )
    return in_maps


class _Runner:
    """Persistent jitted SPMD executor for one built program (one `reps`
    value). Mirrors bass2jax.run_bass_via_pjrt but keeps the jitted
    callable so repeat calls skip retrace/recompile, and lets callers
    pre-place inputs on device for clean timing."""

    def __init__(self, nc):
        import jax
        import numpy as _np
        from jax.sharding import Mesh, NamedSharding, PartitionSpec
        from jax.experimental.shard_map import shard_map
        import concourse.mybir as mybir
        from concourse import bass2jax

        bass2jax.install_neuronx_cc_hook()
        self.jax = jax
        self.nc = nc

        partition_name = (
            nc.partition_id_tensor.name if nc.partition_id_tensor else None
        )
        in_names, out_names, out_avals, zero_outs = [], [], [], []
        for alloc in nc.m.functions[0].allocations:
            if not isinstance(alloc, mybir.MemoryLocationSet):
                continue
            name = alloc.memorylocations[0].name
            if alloc.kind == "ExternalInput":
                if name != partition_name:
                    in_names.append(name)
            elif alloc.kind == "ExternalOutput":
                shape = tuple(alloc.tensor_shape)
                dtype = mybir.dt.np(alloc.dtype)
                out_names.append(name)
                out_avals.append(jax.core.ShapedArray(shape, dtype))
                zero_outs.append(_np.zeros(shape, dtype))
        self.in_names = in_names
        self.out_names = out_names
        self.out_avals = out_avals
        self.zero_outs = zero_outs
        n_params = len(in_names)

        def _body(*args):
            operands = list(args)
            if partition_name is not None:
                operands.append(bass2jax.partition_id_tensor())
            outs = bass2jax._bass_exec_p.bind(
                *operands,
                out_avals=tuple(out_avals),
                in_names=tuple(in_names + out_names + ([partition_name] if partition_name else [])),
                out_names=tuple(out_names),
                lowering_input_output_aliases=(),
                sim_require_finite=True,
                sim_require_nnan=True,
                nc=nc,
            )
            return tuple(outs)

        devices = jax.devices()[:_NCORES]
        self.mesh = Mesh(np.asarray(devices), ("core",))
        self.spec = NamedSharding(self.mesh, PartitionSpec("core"))
        n_outs = len(out_names)
        in_specs = (PartitionSpec("core"),) * (n_params + n_outs)
        out_specs = (PartitionSpec("core"),) * n_outs
        self.fn = jax.jit(
            shard_map(
                _body,
                mesh=self.mesh,
                in_specs=in_specs,
                out_specs=out_specs,
                check_rep=False,
            ),
            keep_unused=True,
        )

    def place_inputs(self, in_maps):
        concat = [
            np.concatenate([np.asarray(m[name]) for m in in_maps], axis=0)
            for name in self.in_names
        ]
        return [self.jax.device_put(a, self.spec) for a in concat]

    def place_zeros(self):
        return [
            self.jax.device_put(
                np.zeros((_NCORES * z.shape[0], *z.shape[1:]), z.dtype), self.spec
            )
            for z in self.zero_outs
        ]

    def __call__(self, dev_inputs, dev_zeros):
        outs = self.fn(*dev_inputs, *dev_zeros)
        self.jax.block_until_ready(outs)
        return outs


_RUNNER_CACHE = {}


def _get_runner(reps: int) -> "_Runner":
    if reps not in _RUNNER_CACHE:
        _RUNNER_CACHE[reps] = _Runner(_get_program(reps))
    return _RUNNER_CACHE[reps]


def _run(in_maps, reps: int):
    r = _get_runner(reps)
    dev_in = r.place_inputs(in_maps)
    dev_z = r.place_zeros()
    t0 = time.perf_counter()
    outs = r(dev_in, dev_z)
    dt = time.perf_counter() - t0
    full = np.asarray(outs[0]).reshape(_NCORES, _IMGS, 2, 128, 1024)
    return full.reshape(_B, _C, 1024), dt


def kernel(x, weight, bias):
    in_maps = _prep_inputs(x, weight, bias)
    reps = int(os.environ.get("BASS_CONV_REPS", "1"))
    out, _ = _run(in_maps, reps)
    return out
